# revision 14
# baseline (speedup 1.0000x reference)
"""BiLSTM-CRF loss kernel for Trainium2 (8 NeuronCores, data-parallel over batch).

v2 design (per core, BL=16 sequences):
  - Forward and backward LSTM *interleaved* in a single loop (f-step i,
    b-step T-1-i): each direction's elementwise chain hides under the
    other direction's weight-load-bound recurrence matmuls.
  - Input projection accumulated directly in PSUM window banks (8-step
    windows, N=128 fills); recurrence matmuls accumulate W_hh*h on top
    (start=False), so no per-step psg+xw add on the critical chain.
  - No per-step masking: forward runs free (suffix padding is never
    consumed); backward zeroes the g-gate window columns once per
    window, which keeps (h,c)=0 exactly through the pad prefix.
  - CRF log-partition (beta recursion, exp space) as a separate tail
    phase with bulk-precomputed exp(emit); 2 independent batch groups
    pipelined. LSTM phase uses only {Sigmoid,Tanh}; tail only {Exp,Ln}
    (one ACT table set each - no per-step table reloads).
  - Embedding stored bf16; gather + DMA-transpose feed the projection
    with zero compute-engine involvement.
  - Gate order host-permuted to [i,f,o,g] so sigmoid gates are
    contiguous per half-bank.
"""

import numpy as np

PAD_IDX = 0
VOCAB, K, E, H = 30000, 20, 256, 256
B, T = 128, 512
NCORES = 8
BL = B // NCORES          # 16 sequences per core
WIN = 8                   # window steps resident in PSUM
NW = T // WIN             # 64 windows
RESCALE = 8               # CRF rescale interval
NG = 2                    # CRF tail groups
GB = BL // NG

_cache = {}


def _build_program():
    from contextlib import ExitStack
    import concourse.bass as bass
    import concourse.bacc as bacc
    import concourse.tile as tile
    from concourse import mybir

    f32 = mybir.dt.float32
    bf16 = mybir.dt.bfloat16
    i32 = mybir.dt.int32
    u8 = mybir.dt.uint8
    AF = mybir.ActivationFunctionType
    OP = mybir.AluOpType

    nc = bacc.Bacc(None, target_bir_lowering=False, debug=False)
    names = {}

    with ExitStack() as ctx:
        tc = ctx.enter_context(tile.TileContext(nc))
        dram = ctx.enter_context(tc.tile_pool(name="dram", bufs=1, space="DRAM"))

        def din(key, shape, dt=f32):
            t = dram.tile(shape, dt, kind="ExternalInput", name=key)
            names[key] = t.tensor.name
            return t

        emb = din("emb", [VOCAB, E], bf16)
        toks = din("toks", [T * BL, 1], i32)          # (w, t, b) t-major
        maskinvu = din("maskinvu", [1, T * BL], u8)   # 1-mask
        xfix = din("xfix", [K, 1])                    # solve(exp(A), 1)
        tags1h = din("tags1h", [K, T * BL], u8)       # one-hot(tag)*mask, t-major
        tagsnx = din("tagsnx", [T * BL, K], u8)       # shifted one-hot*mask, b-major
        tagsfl = din("tagsfl", [T * BL, 1], i32)      # tag ids, b-major
        wih = {d: din(f"wih_{d}", [E, 4 * H], bf16) for d in "fb"}
        whh = {d: din(f"whh_{d}", [E, 4 * H], bf16) for d in "fb"}
        bih = {d: din(f"bih_{d}", [128, 8]) for d in "fb"}
        woutT = din("woutT", [4, 128, K], bf16)       # chunks: Fk0,Fk1,Bk0,Bk1
        bout = din("bout", [K, 1])
        transT = din("transT", [K, K])                # transition.T
        trans = din("trans", [K, K])                  # raw, for row gather
        out_loss = dram.tile([1, BL], f32, kind="ExternalOutput")
        names["out"] = out_loss.tensor.name

        sg = ctx.enter_context(tc.tile_pool(name="sg", bufs=1))
        tmp = ctx.enter_context(tc.tile_pool(name="tmp", bufs=6))
        gat = ctx.enter_context(tc.tile_pool(name="gat", bufs=4))
        xtw = ctx.enter_context(tc.tile_pool(name="xtw", bufs=4))
        fin = ctx.enter_context(tc.tile_pool(name="fin", bufs=3))
        ps_win = ctx.enter_context(tc.tile_pool(name="ps_win", bufs=1, space="PSUM"))
        ps_s = ctx.enter_context(tc.tile_pool(name="ps_s", bufs=4, space="PSUM"))

        # ---- resident SBUF tensors ----
        s_wih = {d: sg.tile([128, 2, 4 * H], bf16, tag=f"wih{d}", name=f"wih{d}") for d in "fb"}
        s_whh = {d: sg.tile([128, 2, 4 * H], bf16, tag=f"whh{d}", name=f"whh{d}") for d in "fb"}
        for d in "fb":
            nc.sync.dma_start(out=s_wih[d][:], in_=wih[d][:].rearrange("(k p) m -> p k m", p=128))
            nc.sync.dma_start(out=s_whh[d][:], in_=whh[d][:].rearrange("(k p) m -> p k m", p=128))
        s_bih = {d: sg.tile([128, 8], f32, tag=f"bih{d}", name=f"bih{d}") for d in "fb"}
        for d in "fb":
            nc.sync.dma_start(out=s_bih[d][:], in_=bih[d][:])
        s_wout = sg.tile([128, 4, K], bf16, tag="wout")
        nc.sync.dma_start(out=s_wout[:], in_=woutT[:].rearrange("c p k -> p c k"))
        s_bout = sg.tile([K, 1], f32, tag="bout")
        nc.sync.dma_start(out=s_bout[:], in_=bout[:])
        s_transT = sg.tile([K, K], f32, tag="transT")
        nc.sync.dma_start(out=s_transT[:], in_=transT[:])

        ones = sg.tile([128, K], f32, tag="ones")
        nc.vector.memset(ones[:], 1.0)
        negone = sg.tile([128, 1], f32, tag="negone")
        nc.vector.memset(negone[:], -1.0)
        twos = sg.tile([128, 1], f32, tag="twos")
        nc.vector.memset(twos[:], 2.0)
        zeros = sg.tile([128, BL], f32, tag="zeros")
        nc.vector.memset(zeros[:], 0.0)

        # mask replicas (uint8): maskinvrep for backward g-gate zeroing
        # (128 partitions); maskinvK for the CRF expE bulk fix (K partitions).
        maskinvK = sg.tile([K, T, BL], u8, tag="maskinvK")
        nc.sync.dma_start(
            out=maskinvK[:],
            in_=bass.AP(tensor=maskinvu.tensor, offset=maskinvu[:].offset,
                        ap=[[0, K], [BL, T], [1, BL]]),
        )
        maskinvrep = sg.tile([128, T, BL], u8, tag="maskinvrep")
        nc.sync.dma_start(
            out=maskinvrep[:],
            in_=bass.AP(tensor=maskinvu.tensor, offset=maskinvu[:].offset,
                        ap=[[0, 128], [BL, T], [1, BL]]),
        )

        emit = sg.tile([K, T, BL], f32, tag="emit")

        NT128 = T * BL // 128   # 64
        idxall = sg.tile([128, NT128], i32, tag="idxall")
        nc.sync.dma_start(out=idxall[:],
                          in_=bass.AP(tensor=toks.tensor, offset=toks[:].offset,
                                      ap=[[1, 128], [128, NT128]]))
        idxtag = sg.tile([128, NT128], i32, tag="idxtag")
        nc.sync.dma_start(out=idxtag[:],
                          in_=bass.AP(tensor=tagsfl.tensor, offset=tagsfl[:].offset,
                                      ap=[[1, 128], [128, NT128]]))
        s_t1h = sg.tile([K, T, BL], u8, tag="s_t1h")
        nc.sync.dma_start(out=s_t1h[:].rearrange("k t b -> k (t b)"), in_=tags1h[:])
        s_tnx = sg.tile([128, NT128, K], u8, tag="s_tnx")
        nc.sync.dma_start(out=s_tnx[:],
                          in_=tagsnx[:].rearrange("(n p) k -> p n k", p=128))

        # LSTM state
        st_h = {d: sg.tile([128, 2, BL], bf16, tag=f"h{d}", name=f"h{d}") for d in "fb"}
        for d in "fb":
            nc.vector.memset(st_h[d][:], 0.0)

        # window PSUM tiles: [128, 8, WIN, BL] f32 = 2 banks per dir.
        # gate chunk order [i,f,o,g]; g-rows host-doubled so one Sigmoid
        # op serves all chunks (tanh(g) = 2*sigmoid(2g)-1).
        win = {d: ps_win.tile([128, 8, WIN, BL], f32, tag=f"win{d}", name=f"win{d}")
               for d in "fb"}

        # craw: per-dir cell state stored as c/2, init 0
        craw = {d: sg.tile([128, 2, BL], f32, tag=f"craw{d}", name=f"craw{d}") for d in "fb"}
        for d in "fb":
            nc.vector.memset(craw[d][:], 0.0)

        # ---- warm-up matmuls ----
        for wt in [s_wih["f"][:, 0, 0:1], s_wih["b"][:, 0, 0:1],
                   s_whh["f"][:, 0, 0:1], s_whh["b"][:, 0, 0:1],
                   s_wout[:, 0, 0:1], ones[:, 0:1]]:
            psd = ps_s.tile([1, 1], f32, tag="pssm")
            nc.tensor.matmul(psd[:], lhsT=wt, rhs=wt, start=True, stop=True)

        def bias_ap(d):
            base = s_bih[d][:, :]
            return bass.AP(tensor=base.tensor, offset=base.offset,
                           ap=[base.ap[0], [1, 8], [0, WIN], [0, BL]])

        def maskinv_ap(t0):
            base = maskinvrep[0:128, t0, 0:BL]
            return bass.AP(tensor=base.tensor, offset=base.offset,
                           ap=[base.ap[0], [0, 2], [BL, WIN], [1, BL]])

        def zeros_ap():
            return bass.AP(tensor=zeros.tensor, offset=zeros[:].offset,
                           ap=[zeros[:].ap[0], [0, 2], [0, WIN], [1, BL]])

        def gather_window(w):
            """Issue embedding gather + transposes for window w -> xT tile."""
            xg = gat.tile([128, E], bf16, tag="xg", name="xg")
            nc.gpsimd.indirect_dma_start(
                out=xg[:], out_offset=None, in_=emb[:],
                in_offset=bass.IndirectOffsetOnAxis(ap=idxall[:, w:w + 1], axis=0),
            )
            xT = xtw.tile([128, 2, 128], bf16, tag="xT", name="xT")
            for k in range(2):
                nc.sync.dma_start_transpose(out=xT[:, k, :], in_=xg[:, k * 128:(k + 1) * 128])
            return xT

        def fill_window(d, w, xT):
            """Project window w for direction d into its PSUM banks."""
            wt = win[d]
            dst = wt[:].rearrange("p m t b -> p m (t b)")
            for m in range(8):
                for k in range(2):
                    nc.tensor.matmul(
                        dst[:, m, :],
                        lhsT=s_wih[d][:, k, m * 128:(m + 1) * 128],
                        rhs=xT[:, k, :],
                        start=(m % 4 == 0 and k == 0), stop=(m % 4 == 3 and k == 1))
            # bias in place (PSUM); does not touch has_written bits
            nc.vector.tensor_tensor(wt[:], wt[:], bias_ap(d), op=OP.add)
            if d == "b":
                # zero g-gate columns at masked (t,b): keeps (h,c)=0 in pad prefix
                nc.vector.copy_predicated(wt[:, 6:8, :, :],
                                          maskinv_ap(w * WIN), zeros_ap())

        def lstm_mm(d, t):
            toff = t % WIN
            wt = win[d]
            for m in range(8):
                for k in range(2):
                    nc.tensor.matmul(
                        wt[:, m, toff, :],
                        lhsT=s_whh[d][:, k, m * 128:(m + 1) * 128],
                        rhs=st_h[d][:, k, :],
                        start=False, stop=(k == 1))

        def emit_mm(d, t):
            """Emission matmul for time t (reads current st_h[d])."""
            pse = ps_s.tile([K, BL], f32, tag="pssm", name=f"pse{d}")
            cbase = 0 if d == "f" else 2
            for k in range(2):
                nc.tensor.matmul(pse[:], lhsT=s_wout[:, cbase + k, :],
                                 rhs=st_h[d][:, k, :], start=(k == 0), stop=(k == 1))
            first = (d == "f") == (t <= (T - 2) // 2)
            if first and d == "f":
                nc.scalar.activation(emit[:, t, :], pse[:], AF.Identity, bias=s_bout[:, 0:1])
            elif first:
                nc.vector.tensor_scalar_add(emit[:, t, :], pse[:], s_bout[:, 0:1])
            else:
                nc.vector.tensor_tensor(emit[:, t, :], pse[:], emit[:, t, :], op=OP.add)

        def lstm_chain(d, t):
            toff = t % WIN
            # one sigmoid over all 8 gate chunks (g-rows pre-doubled);
            # rotating output tile avoids WAR waits on the ACT queue
            sif = tmp.tile([128, 8, BL], f32, tag=f"sif{d}", name=f"sif{d}")
            nc.scalar.activation(sif[:], win[d][:, :, toff, :], AF.Sigmoid)
            # v/2 = (sg-1/2)*si on DVE; u/2 = craw*sf on GpSimd (parallel)
            vv = tmp.tile([128, 2, BL], f32, tag=f"vv{d}", name=f"vv{d}")
            nc.vector.scalar_tensor_tensor(vv[:], sif[:, 6:8, :], 0.5, sif[:, 0:2, :],
                                           op0=OP.subtract, op1=OP.mult)
            uu = tmp.tile([128, 2, BL], f32, tag=f"uu{d}", name=f"uu{d}")
            nc.gpsimd.tensor_tensor(uu[:], craw[d][:], sif[:, 2:4, :], op=OP.mult)
            # craw' = c'/2 = v/2 + u/2
            nc.vector.tensor_tensor(craw[d][:], vv[:], uu[:], op=OP.add)
            th = tmp.tile([128, 2, BL], f32, tag=f"th{d}", name=f"th{d}")
            nc.scalar.activation(th[:], craw[d][:], AF.Tanh, scale=2.0)
            nc.vector.tensor_tensor(st_h[d][:], sif[:, 4:6, :], th[:], op=OP.mult)

        # ---------------- interleaved BiLSTM ----------------
        xT_next = {"f": gather_window(0), "b": gather_window(NW - 1)}
        for i in range(T):
            tf, tb = i, T - 1 - i
            if tf % WIN == 0:
                w = tf // WIN
                fill_window("f", w, xT_next["f"])
                if w + 1 < NW:
                    xT_next["f"] = gather_window(w + 1)
            if i > 0:
                emit_mm("f", tf - 1)
            lstm_mm("f", tf)
            lstm_chain("f", tf)
            if tb % WIN == WIN - 1:
                w = tb // WIN
                fill_window("b", w, xT_next["b"])
                if w - 1 >= 0:
                    xT_next["b"] = gather_window(w - 1)
            if i > 0:
                emit_mm("b", tb + 1)
            lstm_mm("b", tb)
            lstm_chain("b", tb)
        emit_mm("f", T - 1)
        emit_mm("b", 0)

        # ---------------- CRF tail: beta recursion in exp space ----------------
        s_expAT = sg.tile([K, K], f32, tag="expAT")
        nc.scalar.activation(s_expAT[:], s_transT[:], AF.Exp)
        psd = ps_s.tile([1, 1], f32, tag="pssm")
        nc.tensor.matmul(psd[:], lhsT=s_expAT[0:K, 0:1], rhs=s_expAT[0:K, 0:1],
                         start=True, stop=True)

        expE = sg.tile([K, T, BL], f32, tag="expE")
        nc.scalar.activation(expE[:].rearrange("k t b -> k (t b)"),
                             emit[:].rearrange("k t b -> k (t b)"), AF.Exp)
        # bulk fix: expE[masked] = xfix, so A @ (expE*Bv) = Bv exactly in the
        # pad prefix (no per-step predication needed)
        s_xfix = sg.tile([K, 1], f32, tag="s_xfix")
        nc.sync.dma_start(out=s_xfix[:], in_=xfix[:])
        xfix_bc = bass.AP(tensor=s_xfix.tensor, offset=s_xfix[:].offset,
                          ap=[s_xfix[:].ap[0], [0, T * BL]])
        nc.vector.copy_predicated(expE[:].rearrange("k t b -> k (t b)"),
                                  maskinvK[:].rearrange("k t b -> k (t b)"), xfix_bc)

        Lacc = sg.tile([1, BL], f32, tag="Lacc")
        nc.vector.memset(Lacc[:], 0.0)
        # BvSB: periodic correction target; SBvK: running per-column scale
        # (product of all applied rescale factors) replicated over K rows.
        # Every CORR steps, masked columns are reset to SBvK*1s - the exact
        # scaled unit vector - bounding the fp32 drift of the expE-fix path.
        BvSB = sg.tile([K, BL], f32, tag="BvSB")
        nc.vector.memset(BvSB[:], 1.0)
        SBvK = sg.tile([K, BL], f32, tag="SBvK")
        nc.vector.memset(SBvK[:], 1.0)
        CORR = 4

        psb_prev = None
        pend_psr = None
        for t in range(T - 2, -1, -1):
            if psb_prev is None:
                src = BvSB[:]
            elif t % CORR == CORR - 1:
                nc.vector.select(BvSB[:], maskinvK[0:K, t + 1, :], SBvK[:], psb_prev[:])
                src = BvSB[:]
            else:
                src = psb_prev[:]
            bp = tmp.tile([K, BL], f32, tag="bp", name="bp")
            nc.vector.tensor_tensor(bp[:], src, expE[:, t + 1, :], op=OP.mult)
            if pend_psr is not None:
                nc.vector.tensor_tensor(bp[:], bp[:], pend_psr[:], op=OP.mult)
                nc.vector.tensor_tensor(SBvK[:], SBvK[:], pend_psr[:], op=OP.mult)
                pend_psr = None
            psb = ps_s.tile([K, BL], f32, tag="pssm", name="psb")
            nc.tensor.matmul(psb[:], lhsT=s_expAT[:], rhs=bp[:], start=True, stop=True)
            if t % RESCALE == 0 and t > 0:
                # rescale via colsum of bp (any positive per-column scale is
                # exactly compensated through Lacc)
                pss = ps_s.tile([1, BL], f32, tag="pssm", name="pss")
                nc.tensor.matmul(pss[:], lhsT=ones[0:K, 0:1], rhs=bp[:], start=True, stop=True)
                rr = tmp.tile([1, BL], f32, tag="rr")
                nc.vector.reciprocal(rr[:], pss[:])
                psr = ps_s.tile([K, BL], f32, tag="pssm", name="psr")
                nc.tensor.matmul(psr[:], lhsT=ones[0:1, 0:K], rhs=rr[:], start=True, stop=True)
                pend_psr = psr
                lns = tmp.tile([1, BL], f32, tag="lns")
                nc.scalar.activation(lns[:], pss[:], AF.Ln)
                nc.vector.tensor_tensor(Lacc[:], Lacc[:], lns[:], op=OP.add)
            psb_prev = psb

        # ---------------- finalize ----------------
        # logZ = ln(sum_i expE_0 * Bv) + Lacc
        zt = fin.tile([K, BL], f32, tag="zt")
        nc.vector.tensor_tensor(zt[:], psb_prev[:], expE[:, 0, :], op=OP.mult)
        psz = ps_s.tile([1, BL], f32, tag="pssm")
        nc.tensor.matmul(psz[:], lhsT=ones[0:K, 0:1], rhs=zt[:], start=True, stop=True)
        logZ = fin.tile([1, BL], f32, tag="logZ")
        nc.scalar.activation(logZ[:], psz[:], AF.Ln)
        nc.vector.tensor_tensor(logZ[:], logZ[:], Lacc[:], op=OP.add)

        # unary gold score: sum over (k,t) of tags1h * emit, keep b (gpsimd)
        Uacc = fin.tile([K, BL], f32, tag="Uacc")
        nc.vector.memset(Uacc[:], 0.0)
        CH = 32
        TC = T // CH
        for ci in range(CH):
            t1 = fin.tile([K, TC * BL], f32, tag="t1")
            nc.gpsimd.tensor_copy(t1[:], s_t1h[:, ci * TC:(ci + 1) * TC, :].rearrange("p t b -> p (t b)"))
            um = fin.tile([K, TC * BL], f32, tag="um")
            nc.gpsimd.tensor_tensor(
                um[:], t1[:], emit[:, ci * TC:(ci + 1) * TC, :].rearrange("p t b -> p (t b)"),
                op=OP.mult)
            ur = fin.tile([K, BL], f32, tag="ur")
            umr = bass.AP(tensor=um.tensor, offset=um[:].offset,
                          ap=[um[:].ap[0], [1, BL], [BL, TC]])
            nc.vector.tensor_reduce(ur[:], umr, axis=mybir.AxisListType.X, op=OP.add)
            nc.gpsimd.tensor_tensor(Uacc[:], Uacc[:], ur[:], op=OP.add)
        psu = ps_s.tile([1, BL], f32, tag="pssm")
        nc.tensor.matmul(psu[:], lhsT=ones[0:K, 0:1], rhs=Uacc[:], start=True, stop=True)
        score = fin.tile([1, BL], f32, tag="score")
        nc.vector.tensor_copy(score[:], psu[:])

        # transition gold score via row gather
        QT = T // 128
        TRbuf = fin.tile([128, NT128], f32, tag="TRbuf")
        for i in range(NT128):
            tr = gat.tile([128, K], f32, tag="tr")
            nc.gpsimd.indirect_dma_start(
                out=tr[:], out_offset=None, in_=trans[:],
                in_offset=bass.IndirectOffsetOnAxis(ap=idxtag[:, i:i + 1], axis=0))
            sel = gat.tile([128, K], f32, tag="sel")
            nc.gpsimd.tensor_copy(sel[:], s_tnx[:, i, :])
            nc.gpsimd.tensor_tensor(tr[:], tr[:], sel[:], op=OP.mult)
            nc.vector.tensor_reduce(TRbuf[:, i:i + 1], tr[:], axis=mybir.AxisListType.X, op=OP.add)
        pstr = ps_s.tile([1, NT128], f32, tag="pssm")
        nc.tensor.matmul(pstr[:], lhsT=ones[:, 0:1], rhs=TRbuf[:], start=True, stop=True)
        trv = fin.tile([1, BL], f32, tag="trv")
        ptr_ap = bass.AP(tensor=pstr.tensor, offset=pstr[:].offset,
                         ap=[pstr[:].ap[0], [QT, BL], [1, QT]])
        nc.vector.tensor_reduce(trv[:], ptr_ap, axis=mybir.AxisListType.X, op=OP.add)

        # loss = logZ - (score + trans)
        nc.vector.tensor_tensor(score[:], score[:], trv[:], op=OP.add)
        res = fin.tile([1, BL], f32, tag="res")
        nc.vector.tensor_tensor(res[:], logZ[:], score[:], op=OP.subtract)
        nc.sync.dma_start(out=out_loss[:], in_=res[:])

    nc.compile()
    return nc, names


# gate-order permutation: torch [i,f,g,o] -> kernel [i,f,o,g]
_PERM = np.r_[0:512, 768:1024, 512:768]


def _prep_shared(inputs):
    """Host prep of tensors identical across cores."""
    import ml_dtypes
    bf = ml_dtypes.bfloat16
    m = {
        "emb": np.asarray(inputs["embedding"]).astype(bf),
        "bout": np.asarray(inputs["b_out"]).reshape(K, 1).astype(np.float32),
        "transT": np.ascontiguousarray(np.asarray(inputs["transition"]).T).astype(np.float32),
        "trans": np.asarray(inputs["transition"], np.float32),
        "woutT": np.ascontiguousarray(np.asarray(inputs["w_out"]).T.reshape(4, 128, K)).astype(bf),
    }
    # CRF mask-fix vector: exp(A) @ xfix = ones
    A = np.exp(m["trans"].astype(np.float64))
    m["xfix"] = np.linalg.solve(A, np.ones(K)).reshape(K, 1).astype(np.float32)
    for d, sfx in (("f", "_f"), ("b", "_b")):
        w_ih = np.asarray(inputs["w_ih" + sfx]).astype(np.float64)
        w_hh = np.asarray(inputs["w_hh" + sfx]).astype(np.float64)
        bb = np.asarray(inputs["b" + sfx]).astype(np.float64)
        # double g-gate rows: tanh(g) = 2*sigmoid(2g) - 1
        w_ih[512:768] *= 2.0
        w_hh[512:768] *= 2.0
        bb[512:768] *= 2.0
        w_ih, w_hh, bb = w_ih[_PERM], w_hh[_PERM], bb[_PERM]
        m[f"wih_{d}"] = np.ascontiguousarray(w_ih.T).astype(bf)
        m[f"whh_{d}"] = np.ascontiguousarray(w_hh.T).astype(bf)
        m[f"bih_{d}"] = np.ascontiguousarray(bb.reshape(8, 128).T).astype(np.float32)
    return m


def _prep_core(inputs, k, shared):
    s = slice(k * BL, (k + 1) * BL)
    sent = np.asarray(inputs["sentences"][s])          # (16, 512) i32
    tags = np.asarray(inputs["tags"][s])               # (16, 512) i32
    mask = (sent != PAD_IDX)
    # window-major, t-major within window: (w, t, b)
    toks = sent.reshape(BL, NW, WIN).transpose(1, 2, 0).reshape(T * BL, 1)
    oh = (tags[:, :, None] == np.arange(K)[None, None, :])
    tags1h = (oh & mask[:, :, None]).transpose(2, 1, 0).reshape(K, T * BL)
    tnx = np.zeros((BL, T, K), np.float32)
    tnx[:, :-1, :] = (oh[:, 1:, :] & mask[:, 1:, None]).astype(np.float32)
    m = {
        "toks": toks.astype(np.int32),
        "maskinvu": (~mask).T.astype(np.uint8).reshape(1, T * BL),
        "tags1h": tags1h.astype(np.uint8),
        "tagsnx": tnx.reshape(T * BL, K).astype(np.uint8),
        "tagsfl": tags.reshape(T * BL, 1).astype(np.int32),
    }
    m.update(shared)
    return m


def kernel(**inputs):
    from concourse.bass_utils import run_bass_kernel_spmd

    if "prog" not in _cache:
        _cache["prog"] = _build_program()
    nc, names = _cache["prog"]

    shared = _prep_shared(inputs)
    in_maps = []
    for k in range(NCORES):
        m = _prep_core(inputs, k, shared)
        in_maps.append({names[kk]: vv for kk, vv in m.items()})

    res = run_bass_kernel_spmd(nc, in_maps, core_ids=list(range(NCORES)),
                               **_cache.get("run_kwargs", {}))
    out = np.concatenate([r[names["out"]].reshape(BL) for r in res.results])
    _cache["last_results"] = res
    return out.astype(np.float32)


# revision 15
# speedup vs baseline: 1.1475x; 1.1475x over previous
"""BiLSTM-CRF loss kernel for Trainium2 (8 NeuronCores, data-parallel over batch).

v2 design (per core, BL=16 sequences):
  - Forward and backward LSTM *interleaved* in a single loop (f-step i,
    b-step T-1-i): each direction's elementwise chain hides under the
    other direction's weight-load-bound recurrence matmuls.
  - Input projection accumulated directly in PSUM window banks (8-step
    windows, N=128 fills); recurrence matmuls accumulate W_hh*h on top
    (start=False), so no per-step psg+xw add on the critical chain.
  - No per-step masking: forward runs free (suffix padding is never
    consumed); backward zeroes the g-gate window columns once per
    window, which keeps (h,c)=0 exactly through the pad prefix.
  - CRF log-partition (beta recursion, exp space) as a separate tail
    phase with bulk-precomputed exp(emit); 2 independent batch groups
    pipelined. LSTM phase uses only {Sigmoid,Tanh}; tail only {Exp,Ln}
    (one ACT table set each - no per-step table reloads).
  - Embedding stored bf16; gather + DMA-transpose feed the projection
    with zero compute-engine involvement.
  - Gate order host-permuted to [i,f,o,g] so sigmoid gates are
    contiguous per half-bank.
"""

import numpy as np

PAD_IDX = 0
VOCAB, K, E, H = 30000, 20, 256, 256
B, T = 128, 512
NCORES = 8
BL = B // NCORES          # 16 sequences per core
WIN = 8                   # window steps resident in PSUM
NW = T // WIN             # 64 windows
RESCALE = 8               # CRF rescale interval
NG = 2                    # CRF tail groups
GB = BL // NG

_cache = {}


def _build_program():
    from contextlib import ExitStack
    import concourse.bass as bass
    import concourse.bacc as bacc
    import concourse.tile as tile
    from concourse import mybir

    f32 = mybir.dt.float32
    bf16 = mybir.dt.bfloat16
    i32 = mybir.dt.int32
    u8 = mybir.dt.uint8
    AF = mybir.ActivationFunctionType
    OP = mybir.AluOpType

    nc = bacc.Bacc(None, target_bir_lowering=False, debug=False)
    names = {}

    with ExitStack() as ctx:
        tc = ctx.enter_context(tile.TileContext(nc))
        dram = ctx.enter_context(tc.tile_pool(name="dram", bufs=1, space="DRAM"))

        def din(key, shape, dt=f32):
            t = dram.tile(shape, dt, kind="ExternalInput", name=key)
            names[key] = t.tensor.name
            return t

        emb = din("emb", [VOCAB, E], bf16)
        toks = din("toks", [T * BL, 1], i32)          # (w, t, b) t-major
        maskinvu = din("maskinvu", [1, T * BL], u8)   # 1-mask
        xfix = din("xfix", [K, 1])                    # solve(exp(A), 1)
        tags1h = din("tags1h", [K, T * BL], u8)       # one-hot(tag)*mask, t-major
        tagsnx = din("tagsnx", [T * BL, K], u8)       # shifted one-hot*mask, b-major
        tagsfl = din("tagsfl", [T * BL, 1], i32)      # tag ids, b-major
        wih = {d: din(f"wih_{d}", [E, 4 * H], bf16) for d in "fb"}
        whh = {d: din(f"whh_{d}", [E, 4 * H], bf16) for d in "fb"}
        bih = {d: din(f"bih_{d}", [1, 8 * 128], bf16) for d in "fb"}
        woutT = din("woutT", [4, 128, K], bf16)       # chunks: Fk0,Fk1,Bk0,Bk1
        bout = din("bout", [K, 1])
        transT = din("transT", [K, K])                # transition.T
        trans = din("trans", [K, K])                  # raw, for row gather
        out_loss = dram.tile([1, BL], f32, kind="ExternalOutput")
        names["out"] = out_loss.tensor.name

        sg = ctx.enter_context(tc.tile_pool(name="sg", bufs=1))
        tmp = ctx.enter_context(tc.tile_pool(name="tmp", bufs=6))
        gat = ctx.enter_context(tc.tile_pool(name="gat", bufs=4))
        xtw = ctx.enter_context(tc.tile_pool(name="xtw", bufs=4))
        fin = ctx.enter_context(tc.tile_pool(name="fin", bufs=3))
        ps_win = ctx.enter_context(tc.tile_pool(name="ps_win", bufs=1, space="PSUM"))
        ps_s = ctx.enter_context(tc.tile_pool(name="ps_s", bufs=4, space="PSUM"))

        # ---- resident SBUF tensors ----
        s_wih = {d: sg.tile([128, 2, 4 * H], bf16, tag=f"wih{d}", name=f"wih{d}") for d in "fb"}
        s_whh = {d: sg.tile([128, 2, 4 * H], bf16, tag=f"whh{d}", name=f"whh{d}") for d in "fb"}
        for d in "fb":
            nc.sync.dma_start(out=s_wih[d][:], in_=wih[d][:].rearrange("(k p) m -> p k m", p=128))
            nc.sync.dma_start(out=s_whh[d][:], in_=whh[d][:].rearrange("(k p) m -> p k m", p=128))
        s_bihrow = {d: sg.tile([1, 8 * 128], bf16, tag=f"bihrow{d}", name=f"bihrow{d}") for d in "fb"}
        for d in "fb":
            nc.sync.dma_start(out=s_bihrow[d][:], in_=bih[d][:])
        s_wout = sg.tile([128, 4, K], bf16, tag="wout")
        nc.sync.dma_start(out=s_wout[:], in_=woutT[:].rearrange("c p k -> p c k"))
        s_bout = sg.tile([K, 1], f32, tag="bout")
        nc.sync.dma_start(out=s_bout[:], in_=bout[:])
        s_transT = sg.tile([K, K], f32, tag="transT")
        nc.sync.dma_start(out=s_transT[:], in_=transT[:])

        ones = sg.tile([128, K], f32, tag="ones")
        nc.vector.memset(ones[:], 1.0)
        negone = sg.tile([128, 1], f32, tag="negone")
        nc.vector.memset(negone[:], -1.0)
        twos = sg.tile([128, 1], f32, tag="twos")
        nc.vector.memset(twos[:], 2.0)
        zeros = sg.tile([128, BL], f32, tag="zeros")
        nc.vector.memset(zeros[:], 0.0)
        onesrow = sg.tile([1, 128], bf16, tag="onesrow")
        nc.vector.memset(onesrow[:], 1.0)

        # mask replicas (uint8): maskinvrep for backward g-gate zeroing
        # (128 partitions); maskinvK for the CRF expE bulk fix (K partitions).
        maskinvK = sg.tile([K, T, BL], u8, tag="maskinvK")
        nc.sync.dma_start(
            out=maskinvK[:],
            in_=bass.AP(tensor=maskinvu.tensor, offset=maskinvu[:].offset,
                        ap=[[0, K], [BL, T], [1, BL]]),
        )
        maskinvrep = sg.tile([128, T, BL], u8, tag="maskinvrep")
        nc.sync.dma_start(
            out=maskinvrep[:],
            in_=bass.AP(tensor=maskinvu.tensor, offset=maskinvu[:].offset,
                        ap=[[0, 128], [BL, T], [1, BL]]),
        )

        emit = sg.tile([K, T, BL], f32, tag="emit")

        NT128 = T * BL // 128   # 64
        idxall = sg.tile([128, NT128], i32, tag="idxall")
        nc.sync.dma_start(out=idxall[:],
                          in_=bass.AP(tensor=toks.tensor, offset=toks[:].offset,
                                      ap=[[1, 128], [128, NT128]]))
        idxtag = sg.tile([128, NT128], i32, tag="idxtag")
        nc.sync.dma_start(out=idxtag[:],
                          in_=bass.AP(tensor=tagsfl.tensor, offset=tagsfl[:].offset,
                                      ap=[[1, 128], [128, NT128]]))
        s_t1h = sg.tile([K, T, BL], u8, tag="s_t1h")
        nc.sync.dma_start(out=s_t1h[:].rearrange("k t b -> k (t b)"), in_=tags1h[:])
        s_tnx = sg.tile([128, NT128, K], u8, tag="s_tnx")
        nc.sync.dma_start(out=s_tnx[:],
                          in_=tagsnx[:].rearrange("(n p) k -> p n k", p=128))

        # LSTM state
        st_h = {d: sg.tile([128, 2, BL], bf16, tag=f"h{d}", name=f"h{d}") for d in "fb"}
        for d in "fb":
            nc.vector.memset(st_h[d][:], 0.0)

        # window PSUM tiles: [128, 8, WIN, BL] f32 = 2 banks per dir.
        # gate chunk order [i,f,o,g]; g-rows host-doubled so one Sigmoid
        # op serves all chunks (tanh(g) = 2*sigmoid(2g)-1).
        win = {d: ps_win.tile([128, 8, WIN, BL], f32, tag=f"win{d}", name=f"win{d}")
               for d in "fb"}

        # per-dir ping-pong activation/state tiles: [0:8]=sigmoid of gates
        # [i,f,o,g] (written by one ACT op each step), [8:10]=chat state
        # ((c+1)/2; step t writes step t+1's tile, so [sg|chat] stays
        # adjacent for the fused uvh op). Ping-pong kills WAR queue nops.
        sifpp = {d: [sg.tile([128, 10, BL], f32, tag=f"sif{d}{p}", name=f"sif{d}{p}")
                     for p in range(2)] for d in "fb"}
        for d in "fb":
            for p in range(2):
                nc.vector.memset(sifpp[d][p][:, 8:10, :], 0.5)

        # ---- warm-up matmuls ----
        for wt in [s_wih["f"][:, 0, 0:1], s_wih["b"][:, 0, 0:1],
                   s_whh["f"][:, 0, 0:1], s_whh["b"][:, 0, 0:1],
                   s_wout[:, 0, 0:1], ones[:, 0:1]]:
            psd = ps_s.tile([1, 1], f32, tag="pssm")
            nc.tensor.matmul(psd[:], lhsT=wt, rhs=wt, start=True, stop=True)



        def maskinv_ap(t0):
            base = maskinvrep[0:128, t0, 0:BL]
            return bass.AP(tensor=base.tensor, offset=base.offset,
                           ap=[base.ap[0], [0, 2], [BL, WIN], [1, BL]])

        def zeros_ap():
            return bass.AP(tensor=zeros.tensor, offset=zeros[:].offset,
                           ap=[zeros[:].ap[0], [0, 2], [0, WIN], [1, BL]])

        def gather_window(w):
            """Issue embedding gather + transposes for window w -> xT tile."""
            xg = gat.tile([128, E], bf16, tag="xg", name="xg")
            nc.gpsimd.indirect_dma_start(
                out=xg[:], out_offset=None, in_=emb[:],
                in_offset=bass.IndirectOffsetOnAxis(ap=idxall[:, w:w + 1], axis=0),
            )
            xT = xtw.tile([128, 2, 128], bf16, tag="xT", name="xT")
            for k in range(2):
                nc.sync.dma_start_transpose(out=xT[:, k, :], in_=xg[:, k * 128:(k + 1) * 128])
            return xT

        def fill_window(d, w, xT):
            """Project window w for direction d into its PSUM banks."""
            wt = win[d]
            dst = wt[:].rearrange("p m t b -> p m (t b)")
            for m in range(8):
                for k in range(2):
                    nc.tensor.matmul(
                        dst[:, m, :],
                        lhsT=s_wih[d][:, k, m * 128:(m + 1) * 128],
                        rhs=xT[:, k, :],
                        start=(m % 4 == 0 and k == 0), stop=(m % 4 == 3 and k == 1))
            # bias via rank-1 accumulating matmuls (K=1): win[:,m] += b_m x 1
            for m in range(8):
                nc.tensor.matmul(
                    dst[:, m, :],
                    lhsT=s_bihrow[d][0:1, m * 128:(m + 1) * 128],
                    rhs=onesrow[0:1, :],
                    start=False, stop=True)
            if d == "b":
                # zero g-gate columns at masked (t,b): keeps (h,c)=0 in pad prefix
                nc.vector.copy_predicated(wt[:, 6:8, :, :],
                                          maskinv_ap(w * WIN), zeros_ap())

        def lstm_mm(d, t):
            toff = t % WIN
            wt = win[d]
            for m in range(8):
                for k in range(2):
                    nc.tensor.matmul(
                        wt[:, m, toff, :],
                        lhsT=s_whh[d][:, k, m * 128:(m + 1) * 128],
                        rhs=st_h[d][:, k, :],
                        start=False, stop=(k == 1))

        def emit_mm(d, t):
            """Emission matmul for time t (reads current st_h[d])."""
            pse = ps_s.tile([K, BL], f32, tag="pssm", name=f"pse{d}")
            cbase = 0 if d == "f" else 2
            for k in range(2):
                nc.tensor.matmul(pse[:], lhsT=s_wout[:, cbase + k, :],
                                 rhs=st_h[d][:, k, :], start=(k == 0), stop=(k == 1))
            first = (d == "f") == (t <= (T - 2) // 2)
            if first and d == "f":
                nc.scalar.activation(emit[:, t, :], pse[:], AF.Identity, bias=s_bout[:, 0:1])
            elif first:
                nc.vector.tensor_scalar_add(emit[:, t, :], pse[:], s_bout[:, 0:1])
            else:
                nc.vector.tensor_tensor(emit[:, t, :], pse[:], emit[:, t, :], op=OP.add)

        def lstm_chain(d, t):
            toff = t % WIN
            sif = sifpp[d][t % 2]
            sifn = sifpp[d][(t + 1) % 2]
            # one sigmoid over all 8 gate chunks (g-rows pre-doubled)
            nc.scalar.activation(sif[:, 0:8, :], win[d][:, :, toff, :], AF.Sigmoid)
            # uvh = [sg-1/2, chat-1/2] * [si, sf] = [v/2, u/2]
            uvh = tmp.tile([128, 4, BL], f32, tag=f"uvh{d}", name=f"uvh{d}")
            nc.vector.scalar_tensor_tensor(uvh[:], sif[:, 6:10, :], 0.5, sif[:, 0:4, :],
                                           op0=OP.subtract, op1=OP.mult)
            # chat' = v/2 + 1/2 + u/2 -> next step's tile
            nc.vector.scalar_tensor_tensor(sifn[:, 8:10, :], uvh[:, 0:2, :], 0.5,
                                           uvh[:, 2:4, :], op0=OP.add, op1=OP.add)
            th = tmp.tile([128, 2, BL], f32, tag=f"th{d}", name=f"th{d}")
            nc.scalar.activation(th[:], sifn[:, 8:10, :], AF.Tanh,
                                 bias=negone[:, 0:1], scale=2.0)
            nc.vector.tensor_tensor(st_h[d][:], sif[:, 4:6, :], th[:], op=OP.mult)

        # ---------------- interleaved BiLSTM ----------------
        xT_next = {"f": gather_window(0), "b": gather_window(NW - 1)}
        for i in range(T):
            tf, tb = i, T - 1 - i
            if tf % WIN == 0:
                w = tf // WIN
                fill_window("f", w, xT_next["f"])
                if w + 1 < NW:
                    xT_next["f"] = gather_window(w + 1)
            if i > 0:
                emit_mm("f", tf - 1)
            lstm_mm("f", tf)
            lstm_chain("f", tf)
            if tb % WIN == WIN - 1:
                w = tb // WIN
                fill_window("b", w, xT_next["b"])
                if w - 1 >= 0:
                    xT_next["b"] = gather_window(w - 1)
            if i > 0:
                emit_mm("b", tb + 1)
            lstm_mm("b", tb)
            lstm_chain("b", tb)
        emit_mm("f", T - 1)
        emit_mm("b", 0)

        # ---------------- CRF tail: beta recursion in exp space ----------------
        s_expAT = sg.tile([K, K], f32, tag="expAT")
        nc.scalar.activation(s_expAT[:], s_transT[:], AF.Exp)
        psd = ps_s.tile([1, 1], f32, tag="pssm")
        nc.tensor.matmul(psd[:], lhsT=s_expAT[0:K, 0:1], rhs=s_expAT[0:K, 0:1],
                         start=True, stop=True)

        expE = sg.tile([K, T, BL], f32, tag="expE")
        nc.scalar.activation(expE[:].rearrange("k t b -> k (t b)"),
                             emit[:].rearrange("k t b -> k (t b)"), AF.Exp)
        # bulk fix: expE[masked] = xfix, so A @ (expE*Bv) = Bv exactly in the
        # pad prefix (no per-step predication needed)
        s_xfix = sg.tile([K, 1], f32, tag="s_xfix")
        nc.sync.dma_start(out=s_xfix[:], in_=xfix[:])
        xfix_bc = bass.AP(tensor=s_xfix.tensor, offset=s_xfix[:].offset,
                          ap=[s_xfix[:].ap[0], [0, T * BL]])
        nc.vector.copy_predicated(expE[:].rearrange("k t b -> k (t b)"),
                                  maskinvK[:].rearrange("k t b -> k (t b)"), xfix_bc)

        Lacc = sg.tile([1, BL], f32, tag="Lacc")
        nc.vector.memset(Lacc[:], 0.0)
        # BvSB: periodic correction target; SBvK: running per-column scale
        # (product of all applied rescale factors) replicated over K rows.
        # Every CORR steps, masked columns are reset to SBvK*1s - the exact
        # scaled unit vector - bounding the fp32 drift of the expE-fix path.
        BvSB = sg.tile([K, BL], f32, tag="BvSB")
        nc.vector.memset(BvSB[:], 1.0)
        SBvK = sg.tile([K, BL], f32, tag="SBvK")
        nc.vector.memset(SBvK[:], 1.0)
        CORR = 4

        psb_prev = None
        pend_psr = None
        for t in range(T - 2, -1, -1):
            if psb_prev is None:
                src = BvSB[:]
            elif t % CORR == CORR - 1:
                nc.vector.select(BvSB[:], maskinvK[0:K, t + 1, :], SBvK[:], psb_prev[:])
                src = BvSB[:]
            else:
                src = psb_prev[:]
            bp = tmp.tile([K, BL], f32, tag="bp", name="bp")
            nc.vector.tensor_tensor(bp[:], src, expE[:, t + 1, :], op=OP.mult)
            if pend_psr is not None:
                nc.vector.tensor_tensor(bp[:], bp[:], pend_psr[:], op=OP.mult)
                nc.vector.tensor_tensor(SBvK[:], SBvK[:], pend_psr[:], op=OP.mult)
                pend_psr = None
            psb = ps_s.tile([K, BL], f32, tag="pssm", name="psb")
            nc.tensor.matmul(psb[:], lhsT=s_expAT[:], rhs=bp[:], start=True, stop=True)
            if t % RESCALE == 0 and t > 0:
                # rescale via colsum of bp (any positive per-column scale is
                # exactly compensated through Lacc)
                pss = ps_s.tile([1, BL], f32, tag="pssm", name="pss")
                nc.tensor.matmul(pss[:], lhsT=ones[0:K, 0:1], rhs=bp[:], start=True, stop=True)
                rr = tmp.tile([1, BL], f32, tag="rr")
                nc.vector.reciprocal(rr[:], pss[:])
                psr = ps_s.tile([K, BL], f32, tag="pssm", name="psr")
                nc.tensor.matmul(psr[:], lhsT=ones[0:1, 0:K], rhs=rr[:], start=True, stop=True)
                pend_psr = psr
                lns = tmp.tile([1, BL], f32, tag="lns")
                nc.scalar.activation(lns[:], pss[:], AF.Ln)
                nc.vector.tensor_tensor(Lacc[:], Lacc[:], lns[:], op=OP.add)
            psb_prev = psb

        # ---------------- finalize ----------------
        # logZ = ln(sum_i expE_0 * Bv) + Lacc
        zt = fin.tile([K, BL], f32, tag="zt")
        nc.vector.tensor_tensor(zt[:], psb_prev[:], expE[:, 0, :], op=OP.mult)
        psz = ps_s.tile([1, BL], f32, tag="pssm")
        nc.tensor.matmul(psz[:], lhsT=ones[0:K, 0:1], rhs=zt[:], start=True, stop=True)
        logZ = fin.tile([1, BL], f32, tag="logZ")
        nc.scalar.activation(logZ[:], psz[:], AF.Ln)
        nc.vector.tensor_tensor(logZ[:], logZ[:], Lacc[:], op=OP.add)

        # unary gold score: sum over (k,t) of tags1h * emit, keep b (gpsimd)
        Uacc = fin.tile([K, BL], f32, tag="Uacc")
        nc.vector.memset(Uacc[:], 0.0)
        CH = 32
        TC = T // CH
        for ci in range(CH):
            t1 = fin.tile([K, TC * BL], f32, tag="t1")
            nc.gpsimd.tensor_copy(t1[:], s_t1h[:, ci * TC:(ci + 1) * TC, :].rearrange("p t b -> p (t b)"))
            um = fin.tile([K, TC * BL], f32, tag="um")
            nc.gpsimd.tensor_tensor(
                um[:], t1[:], emit[:, ci * TC:(ci + 1) * TC, :].rearrange("p t b -> p (t b)"),
                op=OP.mult)
            ur = fin.tile([K, BL], f32, tag="ur")
            umr = bass.AP(tensor=um.tensor, offset=um[:].offset,
                          ap=[um[:].ap[0], [1, BL], [BL, TC]])
            nc.vector.tensor_reduce(ur[:], umr, axis=mybir.AxisListType.X, op=OP.add)
            nc.gpsimd.tensor_tensor(Uacc[:], Uacc[:], ur[:], op=OP.add)
        psu = ps_s.tile([1, BL], f32, tag="pssm")
        nc.tensor.matmul(psu[:], lhsT=ones[0:K, 0:1], rhs=Uacc[:], start=True, stop=True)
        score = fin.tile([1, BL], f32, tag="score")
        nc.vector.tensor_copy(score[:], psu[:])

        # transition gold score via row gather
        QT = T // 128
        TRbuf = fin.tile([128, NT128], f32, tag="TRbuf")
        for i in range(NT128):
            tr = gat.tile([128, K], f32, tag="tr")
            nc.gpsimd.indirect_dma_start(
                out=tr[:], out_offset=None, in_=trans[:],
                in_offset=bass.IndirectOffsetOnAxis(ap=idxtag[:, i:i + 1], axis=0))
            sel = gat.tile([128, K], f32, tag="sel")
            nc.gpsimd.tensor_copy(sel[:], s_tnx[:, i, :])
            nc.gpsimd.tensor_tensor(tr[:], tr[:], sel[:], op=OP.mult)
            nc.vector.tensor_reduce(TRbuf[:, i:i + 1], tr[:], axis=mybir.AxisListType.X, op=OP.add)
        pstr = ps_s.tile([1, NT128], f32, tag="pssm")
        nc.tensor.matmul(pstr[:], lhsT=ones[:, 0:1], rhs=TRbuf[:], start=True, stop=True)
        trv = fin.tile([1, BL], f32, tag="trv")
        ptr_ap = bass.AP(tensor=pstr.tensor, offset=pstr[:].offset,
                         ap=[pstr[:].ap[0], [QT, BL], [1, QT]])
        nc.vector.tensor_reduce(trv[:], ptr_ap, axis=mybir.AxisListType.X, op=OP.add)

        # loss = logZ - (score + trans)
        nc.vector.tensor_tensor(score[:], score[:], trv[:], op=OP.add)
        res = fin.tile([1, BL], f32, tag="res")
        nc.vector.tensor_tensor(res[:], logZ[:], score[:], op=OP.subtract)
        nc.sync.dma_start(out=out_loss[:], in_=res[:])

    nc.compile()
    return nc, names


# gate-order permutation: torch [i,f,g,o] -> kernel [i,f,o,g]
_PERM = np.r_[0:512, 768:1024, 512:768]


def _prep_shared(inputs):
    """Host prep of tensors identical across cores."""
    import ml_dtypes
    bf = ml_dtypes.bfloat16
    m = {
        "emb": np.asarray(inputs["embedding"]).astype(bf),
        "bout": np.asarray(inputs["b_out"]).reshape(K, 1).astype(np.float32),
        "transT": np.ascontiguousarray(np.asarray(inputs["transition"]).T).astype(np.float32),
        "trans": np.asarray(inputs["transition"], np.float32),
        "woutT": np.ascontiguousarray(np.asarray(inputs["w_out"]).T.reshape(4, 128, K)).astype(bf),
    }
    # CRF mask-fix vector: exp(A) @ xfix = ones
    A = np.exp(m["trans"].astype(np.float64))
    m["xfix"] = np.linalg.solve(A, np.ones(K)).reshape(K, 1).astype(np.float32)
    for d, sfx in (("f", "_f"), ("b", "_b")):
        w_ih = np.asarray(inputs["w_ih" + sfx]).astype(np.float64)
        w_hh = np.asarray(inputs["w_hh" + sfx]).astype(np.float64)
        bb = np.asarray(inputs["b" + sfx]).astype(np.float64)
        # double g-gate rows: tanh(g) = 2*sigmoid(2g) - 1
        w_ih[512:768] *= 2.0
        w_hh[512:768] *= 2.0
        bb[512:768] *= 2.0
        w_ih, w_hh, bb = w_ih[_PERM], w_hh[_PERM], bb[_PERM]
        m[f"wih_{d}"] = np.ascontiguousarray(w_ih.T).astype(bf)
        m[f"whh_{d}"] = np.ascontiguousarray(w_hh.T).astype(bf)
        m[f"bih_{d}"] = np.ascontiguousarray(bb.reshape(1, 8 * 128)).astype(bf)
    return m


def _prep_core(inputs, k, shared):
    s = slice(k * BL, (k + 1) * BL)
    sent = np.asarray(inputs["sentences"][s])          # (16, 512) i32
    tags = np.asarray(inputs["tags"][s])               # (16, 512) i32
    mask = (sent != PAD_IDX)
    # window-major, t-major within window: (w, t, b)
    toks = sent.reshape(BL, NW, WIN).transpose(1, 2, 0).reshape(T * BL, 1)
    oh = (tags[:, :, None] == np.arange(K)[None, None, :])
    tags1h = (oh & mask[:, :, None]).transpose(2, 1, 0).reshape(K, T * BL)
    tnx = np.zeros((BL, T, K), np.float32)
    tnx[:, :-1, :] = (oh[:, 1:, :] & mask[:, 1:, None]).astype(np.float32)
    m = {
        "toks": toks.astype(np.int32),
        "maskinvu": (~mask).T.astype(np.uint8).reshape(1, T * BL),
        "tags1h": tags1h.astype(np.uint8),
        "tagsnx": tnx.reshape(T * BL, K).astype(np.uint8),
        "tagsfl": tags.reshape(T * BL, 1).astype(np.int32),
    }
    m.update(shared)
    return m


def kernel(**inputs):
    from concourse.bass_utils import run_bass_kernel_spmd

    if "prog" not in _cache:
        _cache["prog"] = _build_program()
    nc, names = _cache["prog"]

    shared = _prep_shared(inputs)
    in_maps = []
    for k in range(NCORES):
        m = _prep_core(inputs, k, shared)
        in_maps.append({names[kk]: vv for kk, vv in m.items()})

    res = run_bass_kernel_spmd(nc, in_maps, core_ids=list(range(NCORES)),
                               **_cache.get("run_kwargs", {}))
    out = np.concatenate([r[names["out"]].reshape(BL) for r in res.results])
    _cache["last_results"] = res
    return out.astype(np.float32)


# revision 16
# speedup vs baseline: 1.1699x; 1.0195x over previous
"""BiLSTM-CRF loss kernel for Trainium2 (8 NeuronCores, data-parallel over batch).

v2 design (per core, BL=16 sequences):
  - Forward and backward LSTM *interleaved* in a single loop (f-step i,
    b-step T-1-i): each direction's elementwise chain hides under the
    other direction's weight-load-bound recurrence matmuls.
  - Input projection accumulated directly in PSUM window banks (8-step
    windows, N=128 fills); recurrence matmuls accumulate W_hh*h on top
    (start=False), so no per-step psg+xw add on the critical chain.
  - No per-step masking: forward runs free (suffix padding is never
    consumed); backward zeroes the g-gate window columns once per
    window, which keeps (h,c)=0 exactly through the pad prefix.
  - CRF log-partition (beta recursion, exp space) as a separate tail
    phase with bulk-precomputed exp(emit); 2 independent batch groups
    pipelined. LSTM phase uses only {Sigmoid,Tanh}; tail only {Exp,Ln}
    (one ACT table set each - no per-step table reloads).
  - Embedding stored bf16; gather + DMA-transpose feed the projection
    with zero compute-engine involvement.
  - Gate order host-permuted to [i,f,o,g] so sigmoid gates are
    contiguous per half-bank.
"""

import numpy as np

PAD_IDX = 0
VOCAB, K, E, H = 30000, 20, 256, 256
B, T = 128, 512
NCORES = 8
BL = B // NCORES          # 16 sequences per core
WIN = 8                   # window steps resident in PSUM
NW = T // WIN             # 64 windows
RESCALE = 8               # CRF rescale interval
NG = 2                    # CRF tail groups
GB = BL // NG

_cache = {}


def _build_program():
    from contextlib import ExitStack
    import concourse.bass as bass
    import concourse.bacc as bacc
    import concourse.tile as tile
    from concourse import mybir

    f32 = mybir.dt.float32
    bf16 = mybir.dt.bfloat16
    i32 = mybir.dt.int32
    u8 = mybir.dt.uint8
    AF = mybir.ActivationFunctionType
    OP = mybir.AluOpType

    nc = bacc.Bacc(None, target_bir_lowering=False, debug=False)
    names = {}

    with ExitStack() as ctx:
        tc = ctx.enter_context(tile.TileContext(nc))
        dram = ctx.enter_context(tc.tile_pool(name="dram", bufs=1, space="DRAM"))

        def din(key, shape, dt=f32):
            t = dram.tile(shape, dt, kind="ExternalInput", name=key)
            names[key] = t.tensor.name
            return t

        emb = din("emb", [VOCAB, E], bf16)
        toks = din("toks", [T * BL, 1], i32)          # (w, t, b) t-major
        maskinvu = din("maskinvu", [1, T * BL], u8)   # 1-mask
        xfix = din("xfix", [K, 1])                    # solve(exp(A), 1)
        tags1h = din("tags1h", [K, T * BL], u8)       # one-hot(tag)*mask, t-major
        tagsnx = din("tagsnx", [T * BL, K], u8)       # shifted one-hot*mask, b-major
        tagsfl = din("tagsfl", [T * BL, 1], i32)      # tag ids, b-major
        wih = {d: din(f"wih_{d}", [E, 4 * H], bf16) for d in "fb"}
        whh = {d: din(f"whh_{d}", [E, 4 * H], bf16) for d in "fb"}
        bih = {d: din(f"bih_{d}", [1, 8 * 128], bf16) for d in "fb"}
        woutT = din("woutT", [4, 128, K], bf16)       # chunks: Fk0,Fk1,Bk0,Bk1
        bout = din("bout", [K, 1])
        transT = din("transT", [K, K])                # transition.T
        trans = din("trans", [K, K])                  # raw, for row gather
        out_loss = dram.tile([1, BL], f32, kind="ExternalOutput")
        names["out"] = out_loss.tensor.name

        sg = ctx.enter_context(tc.tile_pool(name="sg", bufs=1))
        tmp = ctx.enter_context(tc.tile_pool(name="tmp", bufs=6))
        gat = ctx.enter_context(tc.tile_pool(name="gat", bufs=4))
        xtw = ctx.enter_context(tc.tile_pool(name="xtw", bufs=4))
        fin = ctx.enter_context(tc.tile_pool(name="fin", bufs=3))
        ps_win = ctx.enter_context(tc.tile_pool(name="ps_win", bufs=1, space="PSUM"))
        ps_s = ctx.enter_context(tc.tile_pool(name="ps_s", bufs=4, space="PSUM"))

        # ---- resident SBUF tensors ----
        s_wih = {d: sg.tile([128, 2, 4 * H], bf16, tag=f"wih{d}", name=f"wih{d}") for d in "fb"}
        s_whh = {d: sg.tile([128, 2, 4 * H], bf16, tag=f"whh{d}", name=f"whh{d}") for d in "fb"}
        for d in "fb":
            nc.sync.dma_start(out=s_wih[d][:], in_=wih[d][:].rearrange("(k p) m -> p k m", p=128))
            nc.sync.dma_start(out=s_whh[d][:], in_=whh[d][:].rearrange("(k p) m -> p k m", p=128))
        s_bihrow = {d: sg.tile([1, 8 * 128], bf16, tag=f"bihrow{d}", name=f"bihrow{d}") for d in "fb"}
        for d in "fb":
            nc.sync.dma_start(out=s_bihrow[d][:], in_=bih[d][:])
        s_wout = sg.tile([128, 4, K], bf16, tag="wout")
        nc.sync.dma_start(out=s_wout[:], in_=woutT[:].rearrange("c p k -> p c k"))
        s_bout = sg.tile([K, 1], f32, tag="bout")
        nc.sync.dma_start(out=s_bout[:], in_=bout[:])
        s_transT = sg.tile([K, K], f32, tag="transT")
        nc.sync.dma_start(out=s_transT[:], in_=transT[:])

        ones = sg.tile([128, K], f32, tag="ones")
        nc.vector.memset(ones[:], 1.0)
        negone = sg.tile([128, 1], f32, tag="negone")
        nc.vector.memset(negone[:], -1.0)
        twos = sg.tile([128, 1], f32, tag="twos")
        nc.vector.memset(twos[:], 2.0)
        zeros = sg.tile([128, BL], f32, tag="zeros")
        nc.vector.memset(zeros[:], 0.0)
        onesrow = sg.tile([1, 128], bf16, tag="onesrow")
        nc.vector.memset(onesrow[:], 1.0)

        # mask replicas (uint8): maskinvrep for backward g-gate zeroing
        # (128 partitions); maskinvK for the CRF expE bulk fix (K partitions).
        maskinvK = sg.tile([K, T, BL], u8, tag="maskinvK")
        nc.sync.dma_start(
            out=maskinvK[:],
            in_=bass.AP(tensor=maskinvu.tensor, offset=maskinvu[:].offset,
                        ap=[[0, K], [BL, T], [1, BL]]),
        )
        maskinvrep = sg.tile([128, T, BL], u8, tag="maskinvrep")
        nc.sync.dma_start(
            out=maskinvrep[:],
            in_=bass.AP(tensor=maskinvu.tensor, offset=maskinvu[:].offset,
                        ap=[[0, 128], [BL, T], [1, BL]]),
        )

        emit = sg.tile([K, T, BL], f32, tag="emit")

        NT128 = T * BL // 128   # 64
        idxall = sg.tile([128, NT128], i32, tag="idxall")
        nc.sync.dma_start(out=idxall[:],
                          in_=bass.AP(tensor=toks.tensor, offset=toks[:].offset,
                                      ap=[[1, 128], [128, NT128]]))
        idxtag = sg.tile([128, NT128], i32, tag="idxtag")
        nc.sync.dma_start(out=idxtag[:],
                          in_=bass.AP(tensor=tagsfl.tensor, offset=tagsfl[:].offset,
                                      ap=[[1, 128], [128, NT128]]))
        s_t1h = sg.tile([K, T, BL], u8, tag="s_t1h")
        nc.sync.dma_start(out=s_t1h[:].rearrange("k t b -> k (t b)"), in_=tags1h[:])
        s_tnx = sg.tile([128, NT128, K], u8, tag="s_tnx")
        nc.sync.dma_start(out=s_tnx[:],
                          in_=tagsnx[:].rearrange("(n p) k -> p n k", p=128))

        # LSTM state
        st_h = {d: sg.tile([128, 2, BL], bf16, tag=f"h{d}", name=f"h{d}") for d in "fb"}
        for d in "fb":
            nc.vector.memset(st_h[d][:], 0.0)

        # window PSUM tiles: [128, 8, WIN, BL] f32 = 2 banks per dir.
        # gate chunk order [i,f,o,g]; g-rows host-doubled so one Sigmoid
        # op serves all chunks (tanh(g) = 2*sigmoid(2g)-1).
        win = {d: ps_win.tile([128, 8, WIN, BL], f32, tag=f"win{d}", name=f"win{d}")
               for d in "fb"}

        # per-dir ping-pong activation/state tiles: [0:8]=sigmoid of gates
        # [i,f,o,g] (written by one ACT op each step), [8:10]=chat state
        # ((c+1)/2; step t writes step t+1's tile, so [sg|chat] stays
        # adjacent for the fused uvh op). Ping-pong kills WAR queue nops.
        sifpp = {d: [sg.tile([128, 10, BL], f32, tag=f"sif{d}{p}", name=f"sif{d}{p}")
                     for p in range(2)] for d in "fb"}
        for d in "fb":
            for p in range(2):
                nc.vector.memset(sifpp[d][p][:, 8:10, :], 0.5)

        # ---- warm-up matmuls ----
        for wt in [s_wih["f"][:, 0, 0:1], s_wih["b"][:, 0, 0:1],
                   s_whh["f"][:, 0, 0:1], s_whh["b"][:, 0, 0:1],
                   s_wout[:, 0, 0:1], ones[:, 0:1]]:
            psd = ps_s.tile([1, 1], f32, tag="pssm")
            nc.tensor.matmul(psd[:], lhsT=wt, rhs=wt, start=True, stop=True)



        def maskinv_ap(t0):
            base = maskinvrep[0:128, t0, 0:BL]
            return bass.AP(tensor=base.tensor, offset=base.offset,
                           ap=[base.ap[0], [0, 2], [BL, WIN], [1, BL]])

        def zeros_ap():
            return bass.AP(tensor=zeros.tensor, offset=zeros[:].offset,
                           ap=[zeros[:].ap[0], [0, 2], [0, WIN], [1, BL]])

        def gather_window(w):
            """Issue embedding gather + transposes for window w -> xT tile."""
            xg = gat.tile([128, E], bf16, tag="xg", name="xg")
            nc.gpsimd.indirect_dma_start(
                out=xg[:], out_offset=None, in_=emb[:],
                in_offset=bass.IndirectOffsetOnAxis(ap=idxall[:, w:w + 1], axis=0),
            )
            xT = xtw.tile([128, 2, 128], bf16, tag="xT", name="xT")
            for k in range(2):
                nc.sync.dma_start_transpose(out=xT[:, k, :], in_=xg[:, k * 128:(k + 1) * 128])
            return xT

        def fill_window(d, w, xT):
            """Project window w for direction d into its PSUM banks."""
            wt = win[d]
            dst = wt[:].rearrange("p m t b -> p m (t b)")
            for m in range(8):
                for k in range(2):
                    nc.tensor.matmul(
                        dst[:, m, :],
                        lhsT=s_wih[d][:, k, m * 128:(m + 1) * 128],
                        rhs=xT[:, k, :],
                        start=(m % 4 == 0 and k == 0), stop=(m % 4 == 3 and k == 1))
            # bias via rank-1 accumulating matmuls (K=1): win[:,m] += b_m x 1
            for m in range(8):
                nc.tensor.matmul(
                    dst[:, m, :],
                    lhsT=s_bihrow[d][0:1, m * 128:(m + 1) * 128],
                    rhs=onesrow[0:1, :],
                    start=False, stop=True)
            if d == "b":
                # zero g-gate columns at masked (t,b): keeps (h,c)=0 in pad prefix
                nc.vector.copy_predicated(wt[:, 6:8, :, :],
                                          maskinv_ap(w * WIN), zeros_ap())

        def lstm_mm(d, t):
            toff = t % WIN
            wt = win[d]
            for m in range(8):
                for k in range(2):
                    nc.tensor.matmul(
                        wt[:, m, toff, :],
                        lhsT=s_whh[d][:, k, m * 128:(m + 1) * 128],
                        rhs=st_h[d][:, k, :],
                        start=False, stop=(k == 1))

        def emit_mm(d, t):
            """Emission matmul for time t (reads current st_h[d])."""
            pse = ps_s.tile([K, BL], f32, tag="pssm", name=f"pse{d}")
            cbase = 0 if d == "f" else 2
            for k in range(2):
                nc.tensor.matmul(pse[:], lhsT=s_wout[:, cbase + k, :],
                                 rhs=st_h[d][:, k, :], start=(k == 0), stop=(k == 1))
            first = (d == "f") == (t <= (T - 2) // 2)
            if first and d == "f":
                nc.scalar.activation(emit[:, t, :], pse[:], AF.Identity, bias=s_bout[:, 0:1])
            elif first:
                nc.vector.tensor_scalar_add(emit[:, t, :], pse[:], s_bout[:, 0:1])
            else:
                nc.vector.tensor_tensor(emit[:, t, :], pse[:], emit[:, t, :], op=OP.add)

        def lstm_chain(d, t):
            toff = t % WIN
            sif = sifpp[d][t % 2]
            sifn = sifpp[d][(t + 1) % 2]
            # one sigmoid over all 8 gate chunks (g-rows pre-doubled)
            nc.scalar.activation(sif[:, 0:8, :], win[d][:, :, toff, :], AF.Sigmoid)
            # uvh = [sg-1/2, chat-1/2] * [si, sf] = [v/2, u/2]
            uvh = tmp.tile([128, 4, BL], f32, tag=f"uvh{d}", name=f"uvh{d}")
            nc.vector.scalar_tensor_tensor(uvh[:], sif[:, 6:10, :], 0.5, sif[:, 0:4, :],
                                           op0=OP.subtract, op1=OP.mult)
            # chat' = v/2 + 1/2 + u/2 -> next step's tile
            nc.vector.scalar_tensor_tensor(sifn[:, 8:10, :], uvh[:, 0:2, :], 0.5,
                                           uvh[:, 2:4, :], op0=OP.add, op1=OP.add)
            th = tmp.tile([128, 2, BL], f32, tag=f"th{d}", name=f"th{d}")
            nc.scalar.activation(th[:], sifn[:, 8:10, :], AF.Tanh,
                                 bias=negone[:, 0:1], scale=2.0)
            nc.vector.tensor_tensor(st_h[d][:], sif[:, 4:6, :], th[:], op=OP.mult)

        # ---------------- interleaved BiLSTM ----------------
        xT_next = {"f": gather_window(0), "b": gather_window(NW - 1)}
        for i in range(T):
            tf, tb = i, T - 1 - i
            if tf % WIN == 0:
                w = tf // WIN
                fill_window("f", w, xT_next["f"])
                if w + 1 < NW:
                    xT_next["f"] = gather_window(w + 1)
            lstm_mm("f", tf)
            if i > 0:
                emit_mm("f", tf - 1)
            lstm_chain("f", tf)
            if tb % WIN == WIN - 1:
                w = tb // WIN
                fill_window("b", w, xT_next["b"])
                if w - 1 >= 0:
                    xT_next["b"] = gather_window(w - 1)
            lstm_mm("b", tb)
            if i > 0:
                emit_mm("b", tb + 1)
            lstm_chain("b", tb)
        emit_mm("f", T - 1)
        emit_mm("b", 0)

        # ---------------- CRF tail: beta recursion in exp space ----------------
        s_expAT = sg.tile([K, K], f32, tag="expAT")
        nc.scalar.activation(s_expAT[:], s_transT[:], AF.Exp)
        psd = ps_s.tile([1, 1], f32, tag="pssm")
        nc.tensor.matmul(psd[:], lhsT=s_expAT[0:K, 0:1], rhs=s_expAT[0:K, 0:1],
                         start=True, stop=True)

        expE = sg.tile([K, T, BL], f32, tag="expE")
        nc.scalar.activation(expE[:].rearrange("k t b -> k (t b)"),
                             emit[:].rearrange("k t b -> k (t b)"), AF.Exp)
        # bulk fix: expE[masked] = xfix, so A @ (expE*Bv) = Bv exactly in the
        # pad prefix (no per-step predication needed)
        s_xfix = sg.tile([K, 1], f32, tag="s_xfix")
        nc.sync.dma_start(out=s_xfix[:], in_=xfix[:])
        xfix_bc = bass.AP(tensor=s_xfix.tensor, offset=s_xfix[:].offset,
                          ap=[s_xfix[:].ap[0], [0, T * BL]])
        nc.vector.copy_predicated(expE[:].rearrange("k t b -> k (t b)"),
                                  maskinvK[:].rearrange("k t b -> k (t b)"), xfix_bc)

        Lacc = sg.tile([1, BL], f32, tag="Lacc")
        nc.vector.memset(Lacc[:], 0.0)
        # BvSB: periodic correction target; SBvK: running per-column scale
        # (product of all applied rescale factors) replicated over K rows.
        # Every CORR steps, masked columns are reset to SBvK*1s - the exact
        # scaled unit vector - bounding the fp32 drift of the expE-fix path.
        BvSB = sg.tile([K, BL], f32, tag="BvSB")
        nc.vector.memset(BvSB[:], 1.0)
        SBvK = sg.tile([K, BL], f32, tag="SBvK")
        nc.vector.memset(SBvK[:], 1.0)
        CORR = 4

        psb_prev = None
        pend_psr = None
        for t in range(T - 2, -1, -1):
            if psb_prev is None:
                src = BvSB[:]
            elif t % CORR == CORR - 1:
                nc.vector.select(BvSB[:], maskinvK[0:K, t + 1, :], SBvK[:], psb_prev[:])
                src = BvSB[:]
            else:
                src = psb_prev[:]
            bp = tmp.tile([K, BL], f32, tag="bp", name="bp")
            nc.vector.tensor_tensor(bp[:], src, expE[:, t + 1, :], op=OP.mult)
            if pend_psr is not None:
                nc.vector.tensor_tensor(bp[:], bp[:], pend_psr[:], op=OP.mult)
                nc.vector.tensor_tensor(SBvK[:], SBvK[:], pend_psr[:], op=OP.mult)
                pend_psr = None
            psb = ps_s.tile([K, BL], f32, tag="pssm", name="psb")
            nc.tensor.matmul(psb[:], lhsT=s_expAT[:], rhs=bp[:], start=True, stop=True)
            if t % RESCALE == 0 and t > 0:
                # rescale via colsum of bp (any positive per-column scale is
                # exactly compensated through Lacc)
                pss = ps_s.tile([1, BL], f32, tag="pssm", name="pss")
                nc.tensor.matmul(pss[:], lhsT=ones[0:K, 0:1], rhs=bp[:], start=True, stop=True)
                rr = tmp.tile([1, BL], f32, tag="rr")
                nc.vector.reciprocal(rr[:], pss[:])
                psr = ps_s.tile([K, BL], f32, tag="pssm", name="psr")
                nc.tensor.matmul(psr[:], lhsT=ones[0:1, 0:K], rhs=rr[:], start=True, stop=True)
                pend_psr = psr
                lns = tmp.tile([1, BL], f32, tag="lns")
                nc.scalar.activation(lns[:], pss[:], AF.Ln)
                nc.vector.tensor_tensor(Lacc[:], Lacc[:], lns[:], op=OP.add)
            psb_prev = psb

        # ---------------- finalize ----------------
        # logZ = ln(sum_i expE_0 * Bv) + Lacc
        zt = fin.tile([K, BL], f32, tag="zt")
        nc.vector.tensor_tensor(zt[:], psb_prev[:], expE[:, 0, :], op=OP.mult)
        psz = ps_s.tile([1, BL], f32, tag="pssm")
        nc.tensor.matmul(psz[:], lhsT=ones[0:K, 0:1], rhs=zt[:], start=True, stop=True)
        logZ = fin.tile([1, BL], f32, tag="logZ")
        nc.scalar.activation(logZ[:], psz[:], AF.Ln)
        nc.vector.tensor_tensor(logZ[:], logZ[:], Lacc[:], op=OP.add)

        # unary gold score: sum over (k,t) of tags1h * emit, keep b (gpsimd)
        Uacc = fin.tile([K, BL], f32, tag="Uacc")
        nc.vector.memset(Uacc[:], 0.0)
        CH = 32
        TC = T // CH
        for ci in range(CH):
            t1 = fin.tile([K, TC * BL], f32, tag="t1")
            nc.gpsimd.tensor_copy(t1[:], s_t1h[:, ci * TC:(ci + 1) * TC, :].rearrange("p t b -> p (t b)"))
            um = fin.tile([K, TC * BL], f32, tag="um")
            nc.gpsimd.tensor_tensor(
                um[:], t1[:], emit[:, ci * TC:(ci + 1) * TC, :].rearrange("p t b -> p (t b)"),
                op=OP.mult)
            ur = fin.tile([K, BL], f32, tag="ur")
            umr = bass.AP(tensor=um.tensor, offset=um[:].offset,
                          ap=[um[:].ap[0], [1, BL], [BL, TC]])
            nc.vector.tensor_reduce(ur[:], umr, axis=mybir.AxisListType.X, op=OP.add)
            nc.gpsimd.tensor_tensor(Uacc[:], Uacc[:], ur[:], op=OP.add)
        psu = ps_s.tile([1, BL], f32, tag="pssm")
        nc.tensor.matmul(psu[:], lhsT=ones[0:K, 0:1], rhs=Uacc[:], start=True, stop=True)
        score = fin.tile([1, BL], f32, tag="score")
        nc.vector.tensor_copy(score[:], psu[:])

        # transition gold score via row gather
        QT = T // 128
        TRbuf = fin.tile([128, NT128], f32, tag="TRbuf")
        for i in range(NT128):
            tr = gat.tile([128, K], f32, tag="tr")
            nc.gpsimd.indirect_dma_start(
                out=tr[:], out_offset=None, in_=trans[:],
                in_offset=bass.IndirectOffsetOnAxis(ap=idxtag[:, i:i + 1], axis=0))
            sel = gat.tile([128, K], f32, tag="sel")
            nc.gpsimd.tensor_copy(sel[:], s_tnx[:, i, :])
            nc.gpsimd.tensor_tensor(tr[:], tr[:], sel[:], op=OP.mult)
            nc.vector.tensor_reduce(TRbuf[:, i:i + 1], tr[:], axis=mybir.AxisListType.X, op=OP.add)
        pstr = ps_s.tile([1, NT128], f32, tag="pssm")
        nc.tensor.matmul(pstr[:], lhsT=ones[:, 0:1], rhs=TRbuf[:], start=True, stop=True)
        trv = fin.tile([1, BL], f32, tag="trv")
        ptr_ap = bass.AP(tensor=pstr.tensor, offset=pstr[:].offset,
                         ap=[pstr[:].ap[0], [QT, BL], [1, QT]])
        nc.vector.tensor_reduce(trv[:], ptr_ap, axis=mybir.AxisListType.X, op=OP.add)

        # loss = logZ - (score + trans)
        nc.vector.tensor_tensor(score[:], score[:], trv[:], op=OP.add)
        res = fin.tile([1, BL], f32, tag="res")
        nc.vector.tensor_tensor(res[:], logZ[:], score[:], op=OP.subtract)
        nc.sync.dma_start(out=out_loss[:], in_=res[:])

    nc.compile()
    return nc, names


# gate-order permutation: torch [i,f,g,o] -> kernel [i,f,o,g]
_PERM = np.r_[0:512, 768:1024, 512:768]


def _prep_shared(inputs):
    """Host prep of tensors identical across cores."""
    import ml_dtypes
    bf = ml_dtypes.bfloat16
    m = {
        "emb": np.asarray(inputs["embedding"]).astype(bf),
        "bout": np.asarray(inputs["b_out"]).reshape(K, 1).astype(np.float32),
        "transT": np.ascontiguousarray(np.asarray(inputs["transition"]).T).astype(np.float32),
        "trans": np.asarray(inputs["transition"], np.float32),
        "woutT": np.ascontiguousarray(np.asarray(inputs["w_out"]).T.reshape(4, 128, K)).astype(bf),
    }
    # CRF mask-fix vector: exp(A) @ xfix = ones
    A = np.exp(m["trans"].astype(np.float64))
    m["xfix"] = np.linalg.solve(A, np.ones(K)).reshape(K, 1).astype(np.float32)
    for d, sfx in (("f", "_f"), ("b", "_b")):
        w_ih = np.asarray(inputs["w_ih" + sfx]).astype(np.float64)
        w_hh = np.asarray(inputs["w_hh" + sfx]).astype(np.float64)
        bb = np.asarray(inputs["b" + sfx]).astype(np.float64)
        # double g-gate rows: tanh(g) = 2*sigmoid(2g) - 1
        w_ih[512:768] *= 2.0
        w_hh[512:768] *= 2.0
        bb[512:768] *= 2.0
        w_ih, w_hh, bb = w_ih[_PERM], w_hh[_PERM], bb[_PERM]
        m[f"wih_{d}"] = np.ascontiguousarray(w_ih.T).astype(bf)
        m[f"whh_{d}"] = np.ascontiguousarray(w_hh.T).astype(bf)
        m[f"bih_{d}"] = np.ascontiguousarray(bb.reshape(1, 8 * 128)).astype(bf)
    return m


def _prep_core(inputs, k, shared):
    s = slice(k * BL, (k + 1) * BL)
    sent = np.asarray(inputs["sentences"][s])          # (16, 512) i32
    tags = np.asarray(inputs["tags"][s])               # (16, 512) i32
    mask = (sent != PAD_IDX)
    # window-major, t-major within window: (w, t, b)
    toks = sent.reshape(BL, NW, WIN).transpose(1, 2, 0).reshape(T * BL, 1)
    oh = (tags[:, :, None] == np.arange(K)[None, None, :])
    tags1h = (oh & mask[:, :, None]).transpose(2, 1, 0).reshape(K, T * BL)
    tnx = np.zeros((BL, T, K), np.float32)
    tnx[:, :-1, :] = (oh[:, 1:, :] & mask[:, 1:, None]).astype(np.float32)
    m = {
        "toks": toks.astype(np.int32),
        "maskinvu": (~mask).T.astype(np.uint8).reshape(1, T * BL),
        "tags1h": tags1h.astype(np.uint8),
        "tagsnx": tnx.reshape(T * BL, K).astype(np.uint8),
        "tagsfl": tags.reshape(T * BL, 1).astype(np.int32),
    }
    m.update(shared)
    return m


def kernel(**inputs):
    from concourse.bass_utils import run_bass_kernel_spmd

    if "prog" not in _cache:
        _cache["prog"] = _build_program()
    nc, names = _cache["prog"]

    shared = _prep_shared(inputs)
    in_maps = []
    for k in range(NCORES):
        m = _prep_core(inputs, k, shared)
        in_maps.append({names[kk]: vv for kk, vv in m.items()})

    res = run_bass_kernel_spmd(nc, in_maps, core_ids=list(range(NCORES)),
                               **_cache.get("run_kwargs", {}))
    out = np.concatenate([r[names["out"]].reshape(BL) for r in res.results])
    _cache["last_results"] = res
    return out.astype(np.float32)
